# revision 37
# baseline (speedup 1.0000x reference)
"""Trainium2 Bass kernel for GroupNorm + single-head spatial self-attention.

Per sample (c=256 channels, n=h*w=1024 tokens):
    xn   = GroupNorm(x, 8 groups) * gw + gb
    qkv  = w_qkv @ xn                       # [768, n]
    sim  = (q^T k) / sqrt(c)                # [n, n]
    attn = softmax(sim, axis=-1)            # (no max-subtraction; |sim/16|<~8)
    out  = w_out @ (attn @ v^T)^T + b_out + x

Sharding: data-parallel over batch, 4 samples per core across 8 cores.

Host-side packing (free, not counted in HW time):
  - "wpack" [128, 2054]: w_qkv^T and w_out^T pre-transposed into the
    [pi, po, o] channel-on-partition layout the PE needs as lhsT, plus
    gn_weight/gn_bias/b_out pre-striped per partition.
  - "x"/"y" [128, 4, 2, 1024]: channel-on-partition layout (c = po*128+pi)
    so every DMA is fully contiguous per partition.

Hardware sync-wait constraints baked into the structure (walrus rejects
anything over): every instruction carries at most ONE semaphore wait.
Tile elides cross-engine waits once the engine has observed that clock,
so tiny same-engine "absorber" ops are inserted to pre-observe fresh
producer ticks before multi-dependency instructions. SBUF tile-slot
claims wait on the previous tile's accessor engines (never elided), so
cross-sample tiles either never rotate (bufs=NSAMP / singles) or have a
single unavoidable claim wait. Each dma_start uses a distinct HW queue
(<=8 total) to avoid queue-reuse waits on the y stores.
"""

import os
import sys
import numpy as np

for _p in ("/opt/trn_rl_repo",):
    if _p not in sys.path and os.path.isdir(_p):
        sys.path.insert(0, _p)

import concourse.bass as bass
import concourse.bacc as bacc_mod
import concourse.mybir as mybir
import concourse.tile as tile
from concourse.masks import make_identity
from concourse.tile_rust import add_dep_helper

P = 128
NSAMP = 4          # samples per core
C = 256            # channels
N = 1024           # tokens (h*w)
NG = 8             # groupnorm groups
GS = C // NG       # 32 channels per group
EPS = 1e-5
FP32 = mybir.dt.float32
MULT = mybir.AluOpType.mult
ADD = mybir.AluOpType.add
SUB = mybir.AluOpType.subtract
WCOLS = 2 * 3 * C + 2 * C + 6   # wT 1536 + woT 512 + striped vectors 6


def build_attention_nc() -> bass.Bass:
    nc = bacc_mod.Bacc()
    x_in = nc.declare_dram_parameter("x", [P, NSAMP, 2, N], FP32, isOutput=False)
    wpack = nc.declare_dram_parameter("wpack", [P, WCOLS], FP32, isOutput=False)
    y_out = nc.declare_dram_parameter("y", [P, NSAMP, 2, N], FP32, isOutput=True)

    with tile.TileContext(nc) as tc:
        _emit(tc, x_in, wpack, y_out)
    nc.finalize()
    return nc


def _emit(tc, x_in, wpack, y_out):
    nc = tc.nc
    from contextlib import ExitStack

    with ExitStack() as ctx:
        singles = ctx.enter_context(tc.tile_pool(name="singles", bufs=1))
        outp = ctx.enter_context(tc.tile_pool(name="outp", bufs=NSAMP))
        work = ctx.enter_context(tc.tile_pool(name="work", bufs=2))
        big = ctx.enter_context(tc.tile_pool(name="big", bufs=1))
        stat = ctx.enter_context(tc.tile_pool(name="stat", bufs=NSAMP))
        pp_mm = ctx.enter_context(tc.tile_pool(name="pp_mm", bufs=5, space="PSUM"))
        pp_tr = ctx.enter_context(tc.tile_pool(name="pp_tr", bufs=2, space="PSUM"))
        pp_jk = ctx.enter_context(tc.tile_pool(name="pp_jk", bufs=1, space="PSUM"))

        # whole-kernel scratch bank: PE wait-absorber outputs and the tiny
        # groupnorm selector-matmul outputs all live in disjoint columns of
        # one PSUM bank, so there is never slot rotation or region WAW
        jk = pp_jk.tile([P, 128], FP32, tag="jk", name="jk")
        jk_idx = [0]

        def jkcol(ncols=1):
            i = jk_idx[0]
            jk_idx[0] += ncols
            assert jk_idx[0] <= 128
            return i

        ident = singles.tile([P, P], FP32)
        make_identity(nc, ident)
        # group selectors: 4 groups of 32 partitions within each po half
        sel = singles.tile([P, 4], FP32)        # sel[pi, j] = 1/GS if pi//GS==j
        nc.gpsimd.memset(sel, 0.0)
        for j in range(4):
            nc.gpsimd.memset(sel[j * GS:(j + 1) * GS, j:j + 1], 1.0 / GS)
        epst = singles.tile([P, 1], FP32)
        nc.vector.memset(epst, EPS)

        # scratch for tiny wait-absorber copies; every use gets a fresh
        # column so no region is ever rewritten
        scr = singles.tile([1, 128], FP32)
        scr_idx = [0]

        def absorb(engine, src_ap):
            i = scr_idx[0]
            scr_idx[0] += 1
            assert i < 128
            if engine == "act":
                return nc.scalar.copy(out=scr[:, i:i + 1], in_=src_ap)
            return nc.vector.tensor_copy(out=scr[:, i:i + 1], in_=src_ap)

        # ---- weights: one DMA, pre-transposed host-side ---------------
        wsb = singles.tile([P, WCOLS], FP32)
        nc.sync.dma_start(wsb, wpack[:, :])                 # queue 0
        wT = wsb[:, 0:1536].rearrange("p (po o) -> p po o", po=2)
        woT = wsb[:, 1536:2048].rearrange("p (po o) -> p po o", po=2)
        gwv = wsb[:, 2048:2050]
        gbv = wsb[:, 2050:2052]
        bvec = wsb[:, 2052:2054]

        # selT[j, pi] = 1 if pi//GS == j (PE transpose of sel); this matmul
        # also pre-observes the gpsimd clock on PE
        selT = singles.tile([4, P], FP32)
        ptsel = pp_tr.tile([P, 512], FP32, tag="tr", name="ptsel")
        nc.tensor.matmul(ptsel[:4, :P], lhsT=sel, rhs=ident, start=True, stop=True)
        nc.vector.tensor_scalar_mul(selT, ptsel[:4, :P], float(GS))
        # pre-observe the wpack DMA tick on PE and DVE
        c = jkcol()
        nc.tensor.matmul(jk[:8, c:c + 1], lhsT=wsb[:, 0:8],
                         rhs=wsb[:, 0:1], start=True, stop=True)
        absorb("dve", wsb[0:1, 2048:2049])

        # ---- x: two pair-loads, fully contiguous ----------------------
        xt = singles.tile([P, NSAMP, 2, N], FP32)
        nc.sync.dma_start(xt[:, 0:2], x_in[:, 0:2])         # queue 1
        nc.sync.dma_start(xt[:, 2:4], x_in[:, 2:4])         # queue 2

        prev_out = None
        for s in range(NSAMP):
            prev_out = _emit_sample(
                tc, s, xt[:, s], y_out, ident, sel, selT, wT, woT, bvec, gwv,
                gbv, epst, jk, jkcol, absorb, prev_out, outp, work, big, stat,
                pp_mm, pp_tr)


def _emit_sample(tc, s, xt, y_out, ident, sel, selT, wT, woT, bvec, gwv, gbv,
                 epst, jk, jkcol, absorb, prev_out, outp, work, big, stat,
                 pp_mm, pp_tr):
    nc = tc.nc
    ts, ds = bass.ts, bass.ds

    # ---- group norm ---------------------------------------------------
    stt = stat.tile([P, 2, 2, 6], FP32, tag="stt")
    for po in range(2):
        for sub in range(2):
            nc.vector.bn_stats(stt[:, po, sub], xt[:, po, ds(sub * 512, 512)])
    mv = stat.tile([P, 2, 2], FP32, tag="mv")  # per-channel (mean, var)
    for po in range(2):
        nc.vector.bn_aggr(mv[:, po], stt[:, po])
    ex2 = stat.tile([P, 2], FP32, tag="ex2")   # E[x^2] per channel
    nc.vector.tensor_tensor(ex2, mv[:, :, 0], mv[:, :, 0], MULT)
    nc.vector.tensor_tensor(ex2, ex2, mv[:, :, 1], ADD)

    gc = jkcol(4)
    gst = jk[:4, gc:gc + 4]                    # [g4, (mean, ex2) x po]
    nc.tensor.matmul(gst[:, 0:2], lhsT=sel, rhs=mv[:, :, 0], start=True, stop=True)
    nc.tensor.matmul(gst[:, 2:4], lhsT=sel, rhs=ex2, start=True, stop=True)

    gss = stat.tile([4, 4], FP32, tag="gss")   # SBUF copy of group stats
    nc.vector.tensor_copy(out=gss, in_=gst)
    grs = stat.tile([4, 4], FP32, tag="grs")   # cols: rstd x po, mean*rstd x po
    tmp = stat.tile([4, 2], FP32, tag="gtmp")
    nc.vector.tensor_tensor(tmp, gss[:, 0:2], gss[:, 0:2], MULT)         # mean^2
    nc.vector.tensor_tensor(tmp, gss[:, 2:4], tmp, SUB)                  # var
    nc.scalar.activation(tmp, tmp, mybir.ActivationFunctionType.Sqrt,
                         bias=epst[:4])
    nc.vector.reciprocal(grs[:, 0:2], tmp)                               # rstd
    nc.vector.tensor_tensor(grs[:, 2:4], gss[:, 0:2], grs[:, 0:2], MULT)

    ac = jkcol(4)
    ab = jk[:, ac:ac + 4]                      # broadcast back to channels
    nc.tensor.matmul(ab, lhsT=selT, rhs=grs, start=True, stop=True)

    acoef = stat.tile([P, 2], FP32, tag="ac")
    bcoef = stat.tile([P, 2], FP32, tag="bc")
    nc.vector.tensor_tensor(acoef, ab[:, 0:2], gwv, MULT)
    nc.vector.tensor_tensor(bcoef, ab[:, 2:4], gwv, MULT)
    nc.vector.tensor_tensor(bcoef, gbv, bcoef, SUB)

    xn = work.tile([P, 2, N], FP32, tag="xn")
    for po in range(2):
        nc.vector.tensor_scalar(xn[:, po], xt[:, po],
                                acoef[:, po:po + 1], bcoef[:, po:po + 1],
                                MULT, ADD)

    # ---- qkv projections ---------------------------------------------
    # q, k in [c, n] layout (o = 0..511); v directly transposed to [n, c]
    qk = big.tile([P, 4, N], FP32, tag="qk", name="qk")
    for mo in range(4):
        ps = [pp_mm.tile([P, 512], FP32, tag="mm", name=f"mm{i}") for i in range(2)]
        for po in range(2):
            for nn in range(2):
                nc.tensor.matmul(ps[nn], lhsT=wT[:, po, ts(mo, P)],
                                 rhs=xn[:, po, ds(nn * 512, 512)],
                                 start=(po == 0), stop=(po == 1))
        if mo == 0:
            ab_i = absorb("act", ps[0][0:1, 0:1])
        for nn in range(2):
            cp = nc.scalar.copy(out=qk[:, mo, ds(nn * 512, 512)], in_=ps[nn])
            if mo == 0 and nn == 0:
                add_dep_helper(cp.ins, ab_i.ins, sync=False, reason="absorb")

    vt = big.tile([P, 8, C], FP32, tag="vt")   # v^T: [token, c]
    for mt in range(8):
        psv = pp_mm.tile([P, 512], FP32, tag="mm", name="mmv")
        for po in range(2):
            nc.tensor.matmul(psv[:, :C], lhsT=xn[:, po, ts(mt, P)],
                             rhs=wT[:, po, ds(512, C)],
                             start=(po == 0), stop=(po == 1))
        if mt == 0:
            ab_i = absorb("act", psv[0:1, 0:1])
        cp = nc.scalar.copy(out=vt[:, mt], in_=psv[:, :C])
        if mt == 0:
            add_dep_helper(cp.ins, ab_i.ins, sync=False, reason="absorb")

    # ---- attention ----------------------------------------------------
    E = big.tile([P, 8, N], FP32, tag="E")         # exp(sim/16), q on parts
    attnT = big.tile([P, 8, N], FP32, tag="attnT")  # normalized, k on parts
    rs2 = stat.tile([P, 8, 2], FP32, tag="rs2")
    rrs = stat.tile([P, 8], FP32, tag="rrs")
    for mq in range(8):
        ps = [pp_mm.tile([P, 512], FP32, tag="mm", name=f"mm{i}") for i in range(2)]
        for po in range(2):
            for nn in range(2):
                nc.tensor.matmul(ps[nn], lhsT=qk[:, po, ts(mq, P)],
                                 rhs=qk[:, 2 + po, ds(nn * 512, 512)],
                                 start=(po == 0), stop=(po == 1))
        if mq == 0 and s > 0:
            abs1 = absorb("act", prev_out[0:1, 0, 0:1])  # late DVE tick of s-1
            abs2 = absorb("act", ps[0][0:1, 0:1])        # PE tick of sim psum
        for nn in range(2):
            ex_i = nc.scalar.activation(E[:, mq, ds(nn * 512, 512)], ps[nn],
                                        mybir.ActivationFunctionType.Exp,
                                        scale=float(C) ** -0.5,
                                        accum_out=rs2[:, mq, nn:nn + 1])
            if mq == 0 and nn == 0 and s > 0:
                add_dep_helper(ex_i.ins, abs1.ins, sync=False, reason="absorb")
                add_dep_helper(ex_i.ins, abs2.ins, sync=False, reason="absorb")
        # softmax normalization folded into E in place (DVE); transposes use
        # the constant identity
        nc.vector.tensor_tensor(rrs[:, mq:mq + 1], rs2[:, mq, 0:1],
                                rs2[:, mq, 1:2], ADD)
        nc.vector.reciprocal(rrs[:, mq:mq + 1], rrs[:, mq:mq + 1])
        # wait-absorber A: pull the ACT (exp) clock onto PE pre-norm
        cA = jkcol()
        jkA = nc.tensor.matmul(jk[:2, cA:cA + 1],
                               lhsT=rs2[:, mq, 0:2],
                               rhs=rs2[:, mq, 0:1], start=True, stop=True)
        nc.vector.tensor_scalar_mul(E[:, mq], E[:, mq], rrs[:, mq:mq + 1])
        # wait-absorber B: pull the DVE (normalize) clock onto PE
        cB = jkcol()
        jkB = nc.tensor.matmul(jk[:8, cB:cB + 1],
                               lhsT=E[:, mq, 508:516],
                               rhs=E[:, mq, 508:509], start=True, stop=True)
        add_dep_helper(jkB.ins, jkA.ins, sync=False, reason="absorber order")
        for half in range(2):
            pt = pp_tr.tile([P, 512], FP32, tag="tr", name="pt")
            for j in range(4):
                ko = half * 4 + j
                tr_i = nc.tensor.matmul(pt[:, ts(j, P)], lhsT=E[:, mq, ts(ko, P)],
                                        rhs=ident, start=True, stop=True)
                if j == 0:
                    add_dep_helper(tr_i.ins, jkB.ins, sync=False, reason="absorb")
            if mq == 0 and half == 0:
                ab_i = absorb("dve", pt[0:1, 0:1])
            cp = nc.vector.tensor_copy(
                out=attnT[:, ds(half * 4, 4), ts(mq, P)],
                in_=pt.rearrange("p (a b) -> p a b", a=4))
            if mq == 0 and half == 0:
                add_dep_helper(cp.ins, ab_i.ins, sync=False, reason="absorb")

    # ---- attn @ v  (out^T in [c, n] layout; reuses the qk slot) -------
    ot = big.tile([P, 2, N], FP32, tag="qk", name="ot")
    for mc in range(2):
        ps = [pp_mm.tile([P, 512], FP32, tag="mm", name=f"mm{i}") for i in range(2)]
        for ko in range(8):
            for nn in range(2):
                nc.tensor.matmul(ps[nn], lhsT=vt[:, ko, ts(mc, P)],
                                 rhs=attnT[:, ko, ds(nn * 512, 512)],
                                 start=(ko == 0), stop=(ko == 7))
        if mc == 0:
            ab_i = absorb("act", ps[0][0:1, 0:1])
        for nn in range(2):
            cp = nc.scalar.copy(out=ot[:, mc, ds(nn * 512, 512)], in_=ps[nn])
            if mc == 0 and nn == 0:
                add_dep_helper(cp.ins, ab_i.ins, sync=False, reason="absorb")

    # ---- output projection + bias + residual -------------------------
    outsb = outp.tile([P, 2, N], FP32, tag="outsb")
    for mo in range(2):
        ps = [pp_mm.tile([P, 512], FP32, tag="mm", name=f"mm{i}") for i in range(2)]
        for po in range(2):
            for nn in range(2):
                nc.tensor.matmul(ps[nn], lhsT=woT[:, po, ts(mo, P)],
                                 rhs=ot[:, po, ds(nn * 512, 512)],
                                 start=(po == 0), stop=(po == 1))
        if mo == 0:
            ab_i = absorb("dve", ps[0][0:1, 0:1])
        for nn in range(2):
            st_i = nc.vector.scalar_tensor_tensor(
                out=outsb[:, mo, ds(nn * 512, 512)], in0=ps[nn],
                scalar=bvec[:, mo:mo + 1], in1=xt[:, mo, ds(nn * 512, 512)],
                op0=ADD, op1=ADD)
            if mo == 0 and nn == 0:
                add_dep_helper(st_i.ins, ab_i.ins, sync=False, reason="absorb")
    nc.sync.dma_start(y_out[:, s], outsb)
    return outsb


def _pack_weights(w_qkv, w_out, gn_weight, gn_bias, b_out):
    wp = np.zeros((P, WCOLS), dtype=np.float32)
    # wT[pi, po, o] = w_qkv[o, po*128+pi]
    wT = w_qkv.T.reshape(2, P, 3 * C).transpose(1, 0, 2)       # [pi, po, o]
    wp[:, 0:1536] = wT.reshape(P, 1536)
    woT = w_out.T.reshape(2, P, C).transpose(1, 0, 2)
    wp[:, 1536:2048] = woT.reshape(P, 512)
    wp[:, 2048:2050] = gn_weight.reshape(2, P).T
    wp[:, 2050:2052] = gn_bias.reshape(2, P).T
    wp[:, 2052:2054] = b_out.reshape(2, P).T
    return wp


_NC_CACHE = None


def _get_nc():
    global _NC_CACHE
    if _NC_CACHE is None:
        _NC_CACHE = build_attention_nc()
    return _NC_CACHE


def make_in_maps(x, gn_weight, gn_bias, w_qkv, w_out, b_out, n_cores=8):
    x = np.asarray(x, dtype=np.float32)
    b = x.shape[0]
    wp = _pack_weights(np.asarray(w_qkv, dtype=np.float32),
                       np.asarray(w_out, dtype=np.float32),
                       np.asarray(gn_weight, dtype=np.float32),
                       np.asarray(gn_bias, dtype=np.float32),
                       np.asarray(b_out, dtype=np.float32))
    # [b, c, n] -> per core [pi, s, po, n]
    x_flat = x.reshape(n_cores, NSAMP, 2, P, N).transpose(0, 3, 1, 2, 4)
    return [
        {"x": np.ascontiguousarray(x_flat[k]), "wpack": wp}
        for k in range(n_cores)
    ]


def unpack_y(results, n_cores=8):
    y = np.stack([results[k]["y"] for k in range(n_cores)], axis=0)
    # [cores, pi, s, po, n] -> [b, c, h, w]
    y = y.transpose(0, 2, 3, 1, 4).reshape(n_cores * NSAMP, C, 32, 32)
    return np.ascontiguousarray(y.astype(np.float32))


def kernel(x, gn_weight, gn_bias, w_qkv, w_out, b_out):
    b, c, h, w = x.shape
    assert (b, c, h * w) == (8 * NSAMP, C, N)
    n_cores = 8
    in_maps = make_in_maps(x, gn_weight, gn_bias, w_qkv, w_out, b_out, n_cores)
    from concourse.bass_utils import run_bass_kernel_spmd

    res = run_bass_kernel_spmd(_get_nc(), in_maps, list(range(n_cores)))
    return unpack_y(res.results, n_cores)
